# revision 1
# baseline (speedup 1.0000x reference)
"""CoAttention kernel for Trainium2 (8 NeuronCores, data-parallel over batch).

Math (per sample): ta = relu(seq_a @ W + b), tb likewise.  The reference
mean-pools the [N, rv_len, M] affinity before softmax, and mean-pooling
commutes with the dot product:

    atob_scores[n, l] = mean_m( ta[n,l,:] . tb_all_tokens[m,:] )
                      = ta[n,l,:] . mean_m( tb_all_tokens[m,:] )

so each side only needs a dot with the *other side's per-sample mean
feature vector* — the 52M-element affinity tensor is never materialized.

Schedule: HBM-bound (12.3 MB of seq per core ~ 40 us at streaming rate),
so the seq stream is issued up front on two dedicated queues (sync=side
b, gpsimd=side a) with all tiles resident — no slot-gated DMAs anywhere.
DMA instruction count is kept minimal (the framework's completion-
semaphore pool is small; recycling inserts cross-engine barriers that
serialize unrelated pipelines): c0+c1 d-chunks ship as one DMA per
sample-side, per-sample score rows and softmax-weight rows each move in
a single reshape DMA, and per-side outputs in one DMA each.

Per-sample tail: PE score matvecs (bank-sized PSUM chunks) -> scalar
copies into one [1, 2560] row -> one DMA into [2RV, RL] softmax layout
-> global-max softmax (masking folded into a fused multiply+accumulate
against the f32 mask, exact because exp(s - global_max) never
underflows here) -> one both-sides weight broadcast (gpsimd) -> the
weighted sum runs as multiply+segmented-reduce split across DVE (side
a) and gpsimd/Pool (side b).
"""
import sys

sys.path.insert(0, "/opt/trn_rl_repo")

import numpy as np

import concourse.bacc as bacc
import concourse.tile as tile
from concourse import mybir

# Problem shape (hardcoded per contest contract)
BZ, RV, RL, DIN, DH = 32, 10, 128, 300, 128
NCORES = 8
BPC = BZ // NCORES            # samples per core: 4
TPC = BPC * RV * RL           # tokens per core per side: 5120
TPS = RV * RL                 # tokens per sample: 1280
RPC = BPC * RV                # reviews per core: 40
NEG_INF = -1e9

f32 = mybir.dt.float32
f32r = mybir.dt.float32r
i32 = mybir.dt.int32
AF = mybir.ActivationFunctionType
AX = mybir.AxisListType
ALU = mybir.AluOpType

# d-chunks of the contraction dim (K <= 128); c0+c1 ship as one DMA
DCH = [(0, 128), (128, 128), (256, 44)]
# free-dim chunks of one sample's tokens (N <= 512, >= 256 for fast fp32r)
NCH = [(0, 512), (512, 512), (1024, 256)]

_CACHE = {}


def _build(iters=1, serial=False, loop_n=0, stage=3):
    nc = bacc.Bacc("TRN2", target_bir_lowering=False, debug=False)

    # Host pre-concatenates d-chunks c0|c1 column-wise so each sample-side
    # ships as one [128, 2*TPS] block of contiguous 10KB rows (plus the
    # [44, TPS] c2 remainder) — plain 2D DMAs with fat descriptors.
    sq01_d = {s: nc.dram_tensor(f"sq01_{s}", [BPC * DH, 2 * TPS], f32r,
                                kind="ExternalInput")
              for s in "ab"}
    sq2_d = {s: nc.dram_tensor(f"sq2_{s}", [BPC * 44, TPS], f32r,
                               kind="ExternalInput")
             for s in "ab"}
    msk2_d = nc.dram_tensor("msk2", [2 * RV, BPC * RL], i32,
                            kind="ExternalInput")
    w_d = nc.dram_tensor("w", [DIN, DH], f32r, kind="ExternalInput")
    bias_d = nc.dram_tensor("bias", [DH, 1], f32, kind="ExternalInput")
    ident_d = nc.dram_tensor("ident", [DH, DH], f32, kind="ExternalInput")

    out_v = {s: nc.dram_tensor(f"out_{s}", [RPC, DH], f32, kind="ExternalOutput")
             for s in "ab"}
    out_w = {s: nc.dram_tensor(f"outw_{s}", [RPC, RL], f32, kind="ExternalOutput")
             for s in "ab"}

    import contextlib
    outer_tc = tile.TileContext(nc) if not serial else None
    with (outer_tc if outer_tc is not None else contextlib.nullcontext()):
      for it_ in range(iters):
        pfx = f"i{it_}_" if iters > 1 else ""
        with (
            tile.TileContext(nc) if serial else contextlib.nullcontext()
        ) as maybe_tc:
          tc = maybe_tc if serial else outer_tc
          with (
            tc.For_i(0, loop_n, 1) if loop_n else contextlib.nullcontext()
          ):
           with (
            tc.tile_pool(name=pfx + "cst", bufs=1) as cst,
            tc.tile_pool(name=pfx + "seq", bufs=8) as seqp,
            tc.tile_pool(name=pfx + "ta", bufs=6) as tap,
            tc.tile_pool(name=pfx + "sm", bufs=1) as smp_pool,
            tc.tile_pool(name=pfx + "ps", bufs=2, space="PSUM") as ps,
        ):
            # constants on the scalar queue, which carries nothing else
            # (relu evictions that unblock the FC pipeline live on scalar,
            # so it must never sit behind big or slot-gated DMAs)
            w_t = {}
            for c, (d0, dw) in enumerate(DCH):
                w_t[c] = cst.tile([dw, DH], f32r, tag=f"w{c}", name=f"{pfx}w_t{c}")
                nc.scalar.dma_start(w_t[c][:], w_d[d0:d0 + dw, :])
            bias_t = cst.tile([DH, 1], f32, tag="bias", name=pfx + "bias_t")
            nc.scalar.dma_start(bias_t[:], bias_d[:])
            mskf = cst.tile([2 * RV, BPC * RL], i32, tag="msk2", name=pfx + "mskf")
            nc.scalar.dma_start(mskf[:], msk2_d[:])
            ident_t = cst.tile([DH, DH], f32, tag="ident", name=pfx + "ident_t")
            nc.scalar.dma_start(ident_t[:], ident_d[:])

            # ---- seq stream: side b on sync, side a on gpsimd; c0+c1 as
            # one [256-row -> [128, 2*TPS]] DMA, c2 [44, TPS] separately.
            # All tiles resident: no dma_start ever waits on a pool slot.
            sq01, sq2 = {}, {}
            for smp in range(BPC):
                for s, q in (("b", nc.sync), ("a", nc.gpsimd)):
                    t01 = seqp.tile([DH, 2 * TPS], f32r, tag="seq01",
                                    name=f"{pfx}sq01_{s}{smp}")
                    q.dma_start(
                        t01[:], sq01_d[s][smp * DH:(smp + 1) * DH, :])
                    sq01[(s, smp)] = t01
                    t2 = seqp.tile([44, TPS], f32r, tag="seq2",
                                   name=f"{pfx}sq2_{s}{smp}")
                    q.dma_start(
                        t2[:], sq2_d[s][smp * 44:(smp + 1) * 44, :])
                    sq2[(s, smp)] = t2

            def sq_rhs(s, smp, c, n0, nw):
                if c < 2:
                    return sq01[(s, smp)][:, c * TPS + n0:c * TPS + n0 + nw]
                return sq2[(s, smp)][:, n0:n0 + nw]

            taT, acc, mean, aoutT = {}, {}, {}, {}
            for s in "ab":
                acc[s] = cst.tile([DH, BPC], f32, tag=f"acc{s}", name=f"{pfx}acc_{s}")
                mean[s] = cst.tile([DH, BPC], f32r, tag=f"mean{s}",
                                   name=f"{pfx}mean_{s}")
                aoutT[s] = cst.tile([DH, RPC], f32, tag=f"aoutT{s}",
                                    name=f"{pfx}aoutT_{s}")
            # softmax weights for all samples side by side (matches mask
            # layout); whole-side output ships as one rearranged DMA
            w2d_full = cst.tile([2 * RV, BPC * RL], f32, tag="w2d",
                                name=pfx + "w2d_full")

            other = {"a": "b", "b": "a"}

            def emit_fc_pair(smp):
                if stage < 1:
                    return
                pfc = {}
                for s in ("b", "a"):
                    pfc[s] = ps.tile([DH, TPS], f32, tag="fc", bufs=2,
                                     name=f"{pfx}pfc_{s}{smp}")
                    taT[(s, smp)] = tap.tile([DH, TPS], f32r, tag="taT",
                                             name=f"{pfx}taT_{s}{smp}")
                # c-outer: 3 weight loads per sample pair instead of 18
                for c in range(3):
                    for s in ("b", "a"):
                        for n0, nw in NCH:
                            nc.tensor.matmul(
                                pfc[s][:, n0:n0 + nw],
                                w_t[c][:],
                                sq_rhs(s, smp, c, n0, nw),
                                start=(c == 0), stop=(c == 2))
                for s in ("b", "a"):
                    nc.scalar.activation(
                        taT[(s, smp)][:], pfc[s][:], AF.Relu,
                        bias=bias_t[:], accum_out=acc[s][:, smp:smp + 1])
                    nc.scalar.mul(mean[s][:, smp:smp + 1],
                                  acc[s][:, smp:smp + 1], 1.0 / TPS)

            def emit_tail(smp):
                if stage < 2:
                    return
                # scores: M=1 matvec against the other side's mean, in
                # bank-sized PSUM chunks -> one [1, 2*TPS] row -> one DMA
                # into the [2RV, RL] softmax layout
                srow = smp_pool.tile([1, 2 * TPS], f32, tag="srow",
                                     name=f"{pfx}srow_{smp}")
                for i, s in enumerate(("a", "b")):
                    for ci, (n0, nw) in enumerate(NCH):
                        pscc = ps.tile([1, 512], f32, tag="sc", bufs=2,
                                       name=f"{pfx}psc_{s}{smp}{ci}")
                        nc.tensor.matmul(
                            pscc[:, :nw],
                            mean[other[s]][:, smp:smp + 1],
                            taT[(s, smp)][:, n0:n0 + nw])
                        nc.scalar.copy(srow[:, i * TPS + n0:i * TPS + n0 + nw],
                                       pscc[:, :nw])
                scs = smp_pool.tile([2 * RV, RL], f32, tag="scs", bufs=2,
                                    name=f"{pfx}scs_{smp}")
                nc.sync.dma_start(scs[:], srow[:])

                # masked softmax (v1-proven op sequence)
                lgs = smp_pool.tile([2 * RV, RL], f32, tag="lgs", bufs=2,
                                    name=f"{pfx}lgs_{smp}")
                nc.vector.memset(lgs[:], NEG_INF)
                nc.vector.copy_predicated(
                    lgs[:], mskf[:, smp * RL:(smp + 1) * RL], scs[:])
                negmax = smp_pool.tile([2 * RV, 1], f32, tag="negmax", bufs=2,
                                       name=f"{pfx}negmax_{smp}")
                nc.vector.reduce_max(out=negmax[:], in_=lgs[:],
                                     axis=AX.X, negate=True)
                e2d = smp_pool.tile([2 * RV, RL], f32, tag="e2d", bufs=2,
                                    name=f"{pfx}e2d_{smp}")
                ssum = smp_pool.tile([2 * RV, 1], f32, tag="ssum", bufs=2,
                                     name=f"{pfx}ssum_{smp}")
                nc.scalar.activation(e2d[:], lgs[:], AF.Exp, bias=negmax[:],
                                     accum_out=ssum[:])
                rec = smp_pool.tile([2 * RV, 1], f32, tag="rec", bufs=2,
                                    name=f"{pfx}rec_{smp}")
                nc.vector.reciprocal(rec[:], ssum[:])
                nc.vector.tensor_scalar_mul(
                    w2d_full[:, smp * RL:(smp + 1) * RL], e2d[:], rec[:])

                if stage < 3:
                    return
                # weighted sums: per-side weight row + broadcast, then
                # multiply + segmented reduce on DVE
                for i, s in enumerate(("a", "b")):
                    wrow = smp_pool.tile([1, TPS], f32, tag="wrow", bufs=2,
                                         name=f"{pfx}wrow_{s}{smp}")
                    nc.gpsimd.dma_start(
                        wrow[:], w2d_full[i * RV:(i + 1) * RV,
                                          smp * RL:(smp + 1) * RL])
                    wbc = smp_pool.tile([DH, TPS], f32, tag="wbc", bufs=2,
                                        name=f"{pfx}wbc_{s}{smp}")
                    nc.gpsimd.partition_broadcast(wbc[:], wrow[:])
                    tmp = smp_pool.tile([DH, TPS], f32, tag="tmp", bufs=2,
                                        name=f"{pfx}tmp_{s}{smp}")
                    nc.vector.tensor_tensor(
                        out=tmp[:], in0=taT[(s, smp)][:].bitcast(f32),
                        in1=wbc[:], op=ALU.mult)
                    nc.vector.reduce_sum(
                        out=aoutT[s][:, smp * RV:(smp + 1) * RV],
                        in_=tmp[:].rearrange("p (r l) -> p r l", r=RV),
                        axis=AX.X)

            # FC runs one sample ahead of the score/softmax/weighted-sum
            # tail so the in-order PE queue never stalls on an eviction.
            for smp in range(BPC):
                emit_fc_pair(smp)
                if smp >= 1:
                    emit_tail(smp - 1)
            emit_tail(BPC - 1)

            # ---- per-side epilogue: weights out, transpose, vectors out
            for si, s in enumerate(("a", "b") if stage >= 2 else ()):
                for smp in range(BPC):
                    nc.sync.dma_start(
                        out_w[s][smp * RV:(smp + 1) * RV, :],
                        w2d_full[si * RV:(si + 1) * RV,
                                 smp * RL:(smp + 1) * RL])
                ptp = ps.tile([RPC, DH], f32, tag="sc", bufs=2,
                              name=f"{pfx}ptp_{s}")
                nc.tensor.matmul(ptp[:], aoutT[s][:], ident_t[:],
                                 is_transpose=True)
                aout = smp_pool.tile([RPC, DH], f32, tag="aout",
                                     name=f"{pfx}aout_{s}")
                nc.vector.tensor_copy(aout[:], ptp[:])
                nc.sync.dma_start(out_v[s][:], aout[:])

    nc.compile()
    return nc


def build_in_maps(seq_a, seq_b, mask_a, mask_b, W, b):
    seq_a = np.asarray(seq_a, dtype=np.float32)
    seq_b = np.asarray(seq_b, dtype=np.float32)
    mask_a = np.asarray(mask_a, dtype=np.int32)
    mask_b = np.asarray(mask_b, dtype=np.int32)
    W = np.asarray(W, dtype=np.float32)
    b = np.asarray(b, dtype=np.float32)

    ident_np = np.eye(DH, dtype=np.float32)
    bias_np = np.ascontiguousarray(b.reshape(DH, 1))
    w_np = np.ascontiguousarray(W)

    in_maps = []
    for core in range(NCORES):
        b0 = core * BPC
        sl = {}
        for name, seq in (("a", seq_a), ("b", seq_b)):
            # [BPC, TPS, DIN] -> [BPC, DIN, TPS]; c0|c1 concatenated
            # column-wise into [BPC*128, 2*TPS], c2 as [BPC*44, TPS]
            chunk = seq[b0:b0 + BPC].reshape(BPC, TPS, DIN).transpose(0, 2, 1)
            c01 = np.concatenate([chunk[:, 0:DH, :], chunk[:, DH:2 * DH, :]],
                                 axis=2)
            sl[f"sq01_{name}"] = np.ascontiguousarray(
                c01.reshape(BPC * DH, 2 * TPS))
            sl[f"sq2_{name}"] = np.ascontiguousarray(
                chunk[:, 2 * DH:DIN, :].reshape(BPC * 44, TPS))
        sl["msk2"] = np.ascontiguousarray(np.concatenate([
            mask[b0:b0 + BPC].reshape(BPC, RV, RL).transpose(1, 0, 2)
            .reshape(RV, BPC * RL) for mask in (mask_a, mask_b)], axis=0))
        sl["w"] = w_np
        sl["bias"] = bias_np
        sl["ident"] = ident_np
        in_maps.append(sl)
    return in_maps


def kernel(seq_a, seq_b, mask_a, mask_b, W, b):
    if "nc" not in _CACHE:
        _CACHE["nc"] = _build()
    nc = _CACHE["nc"]
    in_maps = build_in_maps(seq_a, seq_b, mask_a, mask_b, W, b)

    from concourse.bass_utils import run_bass_kernel_spmd
    res = run_bass_kernel_spmd(nc, in_maps, core_ids=list(range(NCORES)))
    _CACHE["last_result"] = res

    a_out = np.concatenate([r["out_a"] for r in res.results], axis=0)
    b_out = np.concatenate([r["out_b"] for r in res.results], axis=0)
    atob_w = np.concatenate([r["outw_a"] for r in res.results], axis=0)
    btoa_w = np.concatenate([r["outw_b"] for r in res.results], axis=0)
    return (a_out, b_out, atob_w, btoa_w)



# revision 2
# speedup vs baseline: 1.1557x; 1.1557x over previous
"""CoAttention kernel for Trainium2 (8 NeuronCores, data-parallel over batch).

Math (per sample): ta = relu(seq_a @ W + b), tb likewise.  The reference
mean-pools the [N, rv_len, M] affinity before softmax, and mean-pooling
commutes with the dot product:

    atob_scores[n, l] = mean_m( ta[n,l,:] . tb_all_tokens[m,:] )
                      = ta[n,l,:] . mean_m( tb_all_tokens[m,:] )

so each side only needs a dot with the *other side's per-sample mean
feature vector* — the 52M-element affinity tensor is never materialized.

v2 schedule (from the v1 trace: loads done at 55us but tails serialized
to 115us because tail DMAs sat FIFO-behind the bulk seq stream on the
same queues, and gpsimd partition_broadcasts burned 12us):

- seq ships as fp16 (host cast; end-to-end rel-err ~4e-3, tolerance
  2e-2): halves HBM bytes, fp16 matmul runs 1 cyc/row at any N.
- bulk queues carry ONLY bulk: sync = side b c0|c1, gpsimd = side a
  c0|c1 (one [128, 2*TPS] DMA per sample-side).  The c2 remainders
  ([44, TPS] per sample) are batched per side into one fat-row
  [44, BPC*TPS] DMA on the scalar queue.
- ALL small/tail DMAs (score reshape, weight rows, out_w) go on the
  scalar HWDGE queue, which carries no bulk, so a tail never waits for
  the seq stream.
- the [1,TPS]->[128,TPS] weight broadcast is a PE ones-matmul into
  bank-sized PSUM chunks (fp32r, N>=256) instead of gpsimd
  partition_broadcast; DVE multiplies taT(fp16) x wbc(PSUM) into an
  SBUF tmp and one segmented reduce per sample-side produces aoutT.
- emission order never lets a PE instruction wait on data that arrives
  later than its own: FC(s), scores(s) emitted per sample; the
  wbc-broadcast for sample s-2 slots between them.
"""
import sys

sys.path.insert(0, "/opt/trn_rl_repo")

import numpy as np

import concourse.bacc as bacc
import concourse.tile as tile
from concourse import mybir

# Problem shape (hardcoded per contest contract)
BZ, RV, RL, DIN, DH = 32, 10, 128, 300, 128
NCORES = 8
BPC = BZ // NCORES            # samples per core: 4
TPC = BPC * RV * RL           # tokens per core per side: 5120
TPS = RV * RL                 # tokens per sample: 1280
RPC = BPC * RV                # reviews per core: 40
NEG_INF = -1e9

f32 = mybir.dt.float32
f32r = mybir.dt.float32r
f16 = mybir.dt.float16
i32 = mybir.dt.int32
AF = mybir.ActivationFunctionType
AX = mybir.AxisListType
ALU = mybir.AluOpType

# d-chunks of the contraction dim (K <= 128); c0+c1 ship as one DMA
DCH = [(0, 128), (128, 128), (256, 44)]
# free-dim chunks of one sample's tokens (N <= 512 for one PSUM bank)
NCH = [(0, 512), (512, 512), (1024, 256)]

_CACHE = {}


def _build(iters=1, serial=False, loop_n=0, stage=3):
    nc = bacc.Bacc("TRN2", target_bir_lowering=False, debug=False)

    # Host pre-transposes to [DIN, tokens] and casts to fp16; c0|c1
    # concatenated column-wise so each sample-side is one [128, 2*TPS]
    # block of contiguous 5KB rows.  c2 remainders are batched across
    # the 4 samples into one [44, BPC*TPS] block of 10KB rows.
    sq01_d = {s: nc.dram_tensor(f"sq01_{s}", [BPC * DH, 2 * TPS], f16,
                                kind="ExternalInput")
              for s in "ab"}
    sq2_d = {s: nc.dram_tensor(f"sq2_{s}", [44, BPC * TPS], f16,
                               kind="ExternalInput")
             for s in "ab"}
    msk2_d = nc.dram_tensor("msk2", [2 * RV, BPC * RL], i32,
                            kind="ExternalInput")
    w_d = nc.dram_tensor("w", [DIN, DH], f16, kind="ExternalInput")
    bias_d = nc.dram_tensor("bias", [DH, 1], f32, kind="ExternalInput")
    ident_d = nc.dram_tensor("ident", [DH, DH], f32, kind="ExternalInput")

    out_v = {s: nc.dram_tensor(f"out_{s}", [RPC, DH], f32, kind="ExternalOutput")
             for s in "ab"}
    out_w = {s: nc.dram_tensor(f"outw_{s}", [RPC, RL], f32, kind="ExternalOutput")
             for s in "ab"}

    import contextlib
    outer_tc = tile.TileContext(nc) if not serial else None
    with (outer_tc if outer_tc is not None else contextlib.nullcontext()):
      for it_ in range(iters):
        pfx = f"i{it_}_" if iters > 1 else ""
        with (
            tile.TileContext(nc) if serial else contextlib.nullcontext()
        ) as maybe_tc:
          tc = maybe_tc if serial else outer_tc
          with (
            tc.For_i(0, loop_n, 1) if loop_n else contextlib.nullcontext()
          ):
           with (
            tc.tile_pool(name=pfx + "cst", bufs=1) as cst,
            tc.tile_pool(name=pfx + "seq", bufs=8) as seqp,
            tc.tile_pool(name=pfx + "ta", bufs=8) as tap,
            tc.tile_pool(name=pfx + "sm", bufs=1) as smp_pool,
            tc.tile_pool(name=pfx + "ps", bufs=2, space="PSUM") as ps,
        ):
            # ---- constants + c2 batches on the scalar queue (HWDGE,
            # never carries bulk): W first (FC needs it), mask early,
            # then the two c2 batches, ident last (epilogue-only).
            w_t = {}
            for c, (d0, dw) in enumerate(DCH):
                w_t[c] = cst.tile([dw, DH], f16, tag=f"w{c}", name=f"{pfx}w_t{c}")
                nc.scalar.dma_start(w_t[c][:], w_d[d0:d0 + dw, :])
            bias_t = cst.tile([DH, 1], f32, tag="bias", name=pfx + "bias_t")
            nc.scalar.dma_start(bias_t[:], bias_d[:])
            mskf = cst.tile([2 * RV, BPC * RL], i32, tag="msk2", name=pfx + "mskf")
            nc.scalar.dma_start(mskf[:], msk2_d[:])
            sq2 = {}
            for s in "ab":
                sq2[s] = seqp.tile([44, BPC * TPS], f16, tag="seq2",
                                   name=f"{pfx}sq2_{s}")
                nc.scalar.dma_start(sq2[s][:], sq2_d[s][:])
            ident_t = cst.tile([DH, DH], f32, tag="ident", name=pfx + "ident_t")
            nc.scalar.dma_start(ident_t[:], ident_d[:])

            # ---- bulk seq stream: side b on sync, side a on gpsimd;
            # one [128, 2*TPS] fp16 DMA per sample-side, all resident.
            sq01 = {}
            for smp in range(BPC):
                for s, q in (("b", nc.sync), ("a", nc.gpsimd)):
                    t01 = seqp.tile([DH, 2 * TPS], f16, tag="seq01",
                                    name=f"{pfx}sq01_{s}{smp}")
                    q.dma_start(
                        t01[:], sq01_d[s][smp * DH:(smp + 1) * DH, :])
                    sq01[(s, smp)] = t01

            def sq_rhs(s, smp, c, n0, nw):
                if c < 2:
                    return sq01[(s, smp)][:, c * TPS + n0:c * TPS + n0 + nw]
                return sq2[s][:, smp * TPS + n0:smp * TPS + n0 + nw]

            taT, acc, mean16, aoutT = {}, {}, {}, {}
            for s in "ab":
                acc[s] = cst.tile([DH, BPC], f32, tag=f"acc{s}", name=f"{pfx}acc_{s}")
                mean16[s] = cst.tile([DH, BPC], f16, tag=f"mean{s}",
                                     name=f"{pfx}mean_{s}")
                aoutT[s] = cst.tile([DH, RPC], f32, tag=f"aoutT{s}",
                                    name=f"{pfx}aoutT_{s}")
            # softmax weights for all samples side by side (matches mask
            # layout)
            w2d_full = cst.tile([2 * RV, BPC * RL], f32, tag="w2d",
                                name=pfx + "w2d_full")
            ones1 = cst.tile([1, DH], f32, tag="ones", name=pfx + "ones1")
            nc.vector.memset(ones1[:], 1.0)

            other = {"a": "b", "b": "a"}

            def emit_fc_pair(smp):
                if stage < 1:
                    return
                pfc = {}
                for s in ("b", "a"):
                    pfc[s] = ps.tile([DH, TPS], f32, tag="fc", bufs=2,
                                     name=f"{pfx}pfc_{s}{smp}")
                    taT[(s, smp)] = tap.tile([DH, TPS], f16, tag="taT",
                                             name=f"{pfx}taT_{s}{smp}")
                # c-outer: 3 weight loads per sample pair instead of 18
                for c in range(3):
                    for s in ("b", "a"):
                        for n0, nw in NCH:
                            nc.tensor.matmul(
                                pfc[s][:, n0:n0 + nw],
                                w_t[c][:],
                                sq_rhs(s, smp, c, n0, nw),
                                start=(c == 0), stop=(c == 2))
                for s in ("b", "a"):
                    nc.scalar.activation(
                        taT[(s, smp)][:], pfc[s][:], AF.Relu,
                        bias=bias_t[:], accum_out=acc[s][:, smp:smp + 1])
                    nc.scalar.mul(mean16[s][:, smp:smp + 1],
                                  acc[s][:, smp:smp + 1], 1.0 / TPS)

            def emit_scores(smp):
                if stage < 2:
                    return
                # scores: M=1 fp16 matvec against the other side's mean,
                # in bank-sized PSUM chunks -> one [1, 2*TPS] row -> one
                # DMA (scalar queue) into the [2RV, RL] softmax layout
                srow = smp_pool.tile([1, 2 * TPS], f32, tag="srow", bufs=2,
                                     name=f"{pfx}srow_{smp}")
                for i, s in enumerate(("a", "b")):
                    for ci, (n0, nw) in enumerate(NCH):
                        pscc = ps.tile([1, 512], f32, tag="sc", bufs=2,
                                       name=f"{pfx}psc_{s}{smp}{ci}")
                        nc.tensor.matmul(
                            pscc[:, :nw],
                            mean16[other[s]][:, smp:smp + 1],
                            taT[(s, smp)][:, n0:n0 + nw])
                        nc.scalar.copy(srow[:, i * TPS + n0:i * TPS + n0 + nw],
                                       pscc[:, :nw])
                scs = smp_pool.tile([2 * RV, RL], f32, tag="scs", bufs=2,
                                    name=f"{pfx}scs_{smp}")
                nc.scalar.dma_start(scs[:], srow[:])
                return scs

            def emit_soft(smp, scs):
                # masked softmax (v1-proven op sequence), all smalls on
                # the scalar queue
                lgs = smp_pool.tile([2 * RV, RL], f32, tag="lgs", bufs=2,
                                    name=f"{pfx}lgs_{smp}")
                nc.vector.memset(lgs[:], NEG_INF)
                nc.vector.copy_predicated(
                    lgs[:], mskf[:, smp * RL:(smp + 1) * RL], scs[:])
                negmax = smp_pool.tile([2 * RV, 1], f32, tag="negmax", bufs=2,
                                       name=f"{pfx}negmax_{smp}")
                nc.vector.reduce_max(out=negmax[:], in_=lgs[:],
                                     axis=AX.X, negate=True)
                e2d = smp_pool.tile([2 * RV, RL], f32, tag="e2d", bufs=2,
                                    name=f"{pfx}e2d_{smp}")
                ssum = smp_pool.tile([2 * RV, 1], f32, tag="ssum", bufs=2,
                                     name=f"{pfx}ssum_{smp}")
                nc.scalar.activation(e2d[:], lgs[:], AF.Exp, bias=negmax[:],
                                     accum_out=ssum[:])
                rec = smp_pool.tile([2 * RV, 1], f32, tag="rec", bufs=2,
                                    name=f"{pfx}rec_{smp}")
                nc.vector.reciprocal(rec[:], ssum[:])
                nc.vector.tensor_scalar_mul(
                    w2d_full[:, smp * RL:(smp + 1) * RL], e2d[:], rec[:])
                # ship this sample's softmax weights + flatten per-side
                # weight rows (scalar queue; all small)
                wrow = {}
                for i, s in enumerate(("a", "b")):
                    nc.scalar.dma_start(
                        out_w[s][smp * RV:(smp + 1) * RV, :],
                        w2d_full[i * RV:(i + 1) * RV,
                                 smp * RL:(smp + 1) * RL])
                    wrow[s] = smp_pool.tile([1, TPS], f32, tag=f"wrow{s}",
                                            bufs=2, name=f"{pfx}wrow_{s}{smp}")
                    nc.scalar.dma_start(
                        wrow[s][:], w2d_full[i * RV:(i + 1) * RV,
                                             smp * RL:(smp + 1) * RL])
                return wrow

            def emit_wsum(smp, wrow):
                if stage < 3:
                    return
                # weighted sums: PE ones-matmul broadcasts the weight row
                # into bank-sized PSUM chunks; DVE multiplies with
                # taT(fp16) into an SBUF tmp; one segmented reduce per
                # sample-side
                for i, s in enumerate(("a", "b")):
                    tmp = smp_pool.tile([DH, TPS], f32, tag="tmp", bufs=2,
                                        name=f"{pfx}tmp_{s}{smp}")
                    for ci, (n0, nw) in enumerate(NCH):
                        wbc = ps.tile([DH, 512], f32, tag="sc", bufs=2,
                                      name=f"{pfx}wbc_{s}{smp}{ci}")
                        nc.tensor.matmul(
                            wbc[:, :nw],
                            ones1[:].bitcast(f32r),
                            wrow[s][:, n0:n0 + nw].bitcast(f32r))
                        nc.vector.tensor_tensor(
                            out=tmp[:, n0:n0 + nw], in0=taT[(s, smp)][:, n0:n0 + nw],
                            in1=wbc[:, :nw], op=ALU.mult)
                    nc.vector.reduce_sum(
                        out=aoutT[s][:, smp * RV:(smp + 1) * RV],
                        in_=tmp[:].rearrange("p (r l) -> p r l", r=RV),
                        axis=AX.X)

            # Emission order: FC(s) + scores(s) per sample; softmax(s-1)
            # (no PE work) right after; wbc-broadcast(s-2) between FC
            # groups so no PE instruction ever waits on data that
            # arrives later than its own sample.
            scs_t, wrow_t = {}, {}
            for smp in range(BPC):
                emit_fc_pair(smp)
                scs_t[smp] = emit_scores(smp)
                if stage >= 2 and smp >= 2:
                    emit_wsum(smp - 2, wrow_t[smp - 2])
                if stage >= 2 and smp >= 1:
                    wrow_t[smp - 1] = emit_soft(smp - 1, scs_t[smp - 1])
            if stage >= 2:
                wrow_t[BPC - 1] = emit_soft(BPC - 1, scs_t[BPC - 1])
                emit_wsum(BPC - 2, wrow_t[BPC - 2])
                emit_wsum(BPC - 1, wrow_t[BPC - 1])

            # ---- per-side epilogue: transpose aoutT, vectors out (sync
            # queue is idle by now)
            for si, s in enumerate(("a", "b") if stage >= 3 else ()):
                ptp = ps.tile([RPC, DH], f32, tag="sc", bufs=2,
                              name=f"{pfx}ptp_{s}")
                nc.tensor.matmul(ptp[:], aoutT[s][:], ident_t[:],
                                 is_transpose=True)
                aout = smp_pool.tile([RPC, DH], f32, tag="aout",
                                     name=f"{pfx}aout_{s}")
                nc.vector.tensor_copy(aout[:], ptp[:])
                nc.sync.dma_start(out_v[s][:], aout[:])

    nc.compile()
    return nc


def build_in_maps(seq_a, seq_b, mask_a, mask_b, W, b):
    seq_a = np.asarray(seq_a, dtype=np.float32)
    seq_b = np.asarray(seq_b, dtype=np.float32)
    mask_a = np.asarray(mask_a, dtype=np.int32)
    mask_b = np.asarray(mask_b, dtype=np.int32)
    W = np.asarray(W, dtype=np.float32)
    b = np.asarray(b, dtype=np.float32)

    ident_np = np.eye(DH, dtype=np.float32)
    bias_np = np.ascontiguousarray(b.reshape(DH, 1))
    w_np = np.ascontiguousarray(W.astype(np.float16))

    in_maps = []
    for core in range(NCORES):
        b0 = core * BPC
        sl = {}
        for name, seq in (("a", seq_a), ("b", seq_b)):
            # [BPC, TPS, DIN] -> [BPC, DIN, TPS] fp16; c0|c1 concatenated
            # column-wise into [BPC*128, 2*TPS]; c2 batched across the 4
            # samples into [44, BPC*TPS]
            chunk = (seq[b0:b0 + BPC].reshape(BPC, TPS, DIN)
                     .transpose(0, 2, 1).astype(np.float16))
            c01 = np.concatenate([chunk[:, 0:DH, :], chunk[:, DH:2 * DH, :]],
                                 axis=2)
            sl[f"sq01_{name}"] = np.ascontiguousarray(
                c01.reshape(BPC * DH, 2 * TPS))
            sl[f"sq2_{name}"] = np.ascontiguousarray(
                chunk[:, 2 * DH:DIN, :].transpose(1, 0, 2)
                .reshape(44, BPC * TPS))
        sl["msk2"] = np.ascontiguousarray(np.concatenate([
            mask[b0:b0 + BPC].reshape(BPC, RV, RL).transpose(1, 0, 2)
            .reshape(RV, BPC * RL) for mask in (mask_a, mask_b)], axis=0))
        sl["w"] = w_np
        sl["bias"] = bias_np
        sl["ident"] = ident_np
        in_maps.append(sl)
    return in_maps


def kernel(seq_a, seq_b, mask_a, mask_b, W, b):
    if "nc" not in _CACHE:
        _CACHE["nc"] = _build()
    nc = _CACHE["nc"]
    in_maps = build_in_maps(seq_a, seq_b, mask_a, mask_b, W, b)

    from concourse.bass_utils import run_bass_kernel_spmd
    res = run_bass_kernel_spmd(nc, in_maps, core_ids=list(range(NCORES)))
    _CACHE["last_result"] = res

    a_out = np.concatenate([r["out_a"] for r in res.results], axis=0)
    b_out = np.concatenate([r["out_b"] for r in res.results], axis=0)
    atob_w = np.concatenate([r["outw_a"] for r in res.results], axis=0)
    btoa_w = np.concatenate([r["outw_b"] for r in res.results], axis=0)
    return (a_out, b_out, atob_w, btoa_w)
